# revision 2
# baseline (speedup 1.0000x reference)
"""Causal self-attention (B=2, S=2048, E=1024, H=16) on 8 TRN2 NeuronCores.

Sharding: cores 0-3 own batch 0, cores 4-7 own batch 1; within a batch
group each core owns 4 heads (tensor parallel). Per core:
  1. QKV projection for its 4 heads, computed transposed ([feat, seq]) so
     both operands of the scores matmul come out in [head_dim, seq] layout.
     V is computed untransposed ([seq, feat]) with an extra ones column
     (softmax denominator trick).
  2. Causal attention with scores computed TRANSPOSED ([k, q] layout):
     probsT = exp(scoresT/8 (+ mask on diagonal blocks)), then
     out[q, d] = probsT.T @ [v | 1] contracts k on the partition axis --
     no probability-matrix transpose, no row-max pass (scores are O(4)).
  3. PE-transpose of the normalized attention output, AllToAll across all
     8 cores (each core ends up owning 256 rows of EACH batch with the
     full E dimension), local output projection, disjoint row writes.
Biases: q/k bias via per-partition tensor_scalar add; v bias folded into
the output-projection bias on the host (softmax rows sum to 1); output
bias via a rank-1 (K=1) accumulating matmul.
All matmuls in bf16 with fp32 PSUM accumulation; final output fp32.
"""

import numpy as np
import ml_dtypes

BF16 = ml_dtypes.bfloat16

B, S, E = 2, 2048, 1024
H, D = 16, 64
NCORES = 8
HPC = 4            # heads per core
HB = HPC * D       # 256, E-slice per core
SB = S // NCORES   # 256 rows of each batch owned per core after A2A
QT = S // 128      # 16 q tiles
KT = E // 128      # 8 contraction tiles over E
MASK_NEG = -240.0  # exp(0.125 * -240) = e^-30

_CACHE = {}


def _build():
    import concourse.bass as bass
    import concourse.bacc as bacc
    import concourse.mybir as mybir
    import concourse.tile as tile

    f32 = mybir.dt.float32
    bf16 = mybir.dt.bfloat16
    AF = mybir.ActivationFunctionType

    nc = bacc.Bacc("TRN2", target_bir_lowering=False, debug=False,
                   num_devices=NCORES)

    # ---- per-core external I/O (each core receives its own shard) ----
    xT = nc.dram_tensor("xT", [E, S], bf16, kind="ExternalInput")          # x[b].T
    wqkT = nc.dram_tensor("wqkT", [E, 2 * HB], bf16, kind="ExternalInput")  # W_in.T cols [q|k] for 4 heads
    wv1T = nc.dram_tensor("wv1T", [E, HPC * 65], bf16, kind="ExternalInput")  # interleaved [v_h(64)|0] x4
    bqk = nc.dram_tensor("bqk", [2 * HB, 1], f32, kind="ExternalInput")
    bv1 = nc.dram_tensor("bv1", [1, HPC * 65], bf16, kind="ExternalInput")   # ones at 65h+64
    woT = nc.dram_tensor("woT", [E, E], bf16, kind="ExternalInput")          # W_out.T
    bo = nc.dram_tensor("bo", [1, E], bf16, kind="ExternalInput")            # b_out + W_out @ b_v
    maskT = nc.dram_tensor("maskT", [128, 128], f32, kind="ExternalInput")   # 0 / -240 (k>q)
    ident = nc.dram_tensor("ident", [128, 128], bf16, kind="ExternalInput")
    y = nc.dram_tensor("y", [2 * SB, E], f32, kind="ExternalOutput")

    # ---- internal DRAM bounce buffers for the AllToAll ----
    a2a_in = nc.dram_tensor("a2a_in", [NCORES * HB, SB], bf16)
    a2a_out = nc.dram_tensor("a2a_out", [NCORES * HB, SB], bf16)

    with tile.TileContext(nc) as tc:
        from contextlib import ExitStack
        with ExitStack() as ctx:
            P = 128
            persist = ctx.enter_context(tc.tile_pool(name="persist", bufs=1))
            mm = ctx.enter_context(tc.tile_pool(name="mm", bufs=4, space="PSUM"))
            ps = ctx.enter_context(tc.tile_pool(name="ps", bufs=2, space="PSUM"))
            po = ctx.enter_context(tc.tile_pool(name="po", bufs=2, space="PSUM"))
            sbs = ctx.enter_context(tc.tile_pool(name="sbs", bufs=2))
            probs = ctx.enter_context(tc.tile_pool(name="probs", bufs=3))
            aop = ctx.enter_context(tc.tile_pool(name="aop", bufs=2))
            otp = ctx.enter_context(tc.tile_pool(name="otp", bufs=2))
            rcp = ctx.enter_context(tc.tile_pool(name="rcp", bufs=2))
            yp = ctx.enter_context(tc.tile_pool(name="yp", bufs=2))

            def pt(name, shape, dt):
                t = persist.tile(shape, dt, tag=name)
                return t

            # ---- persistent SBUF loads ----
            xTs = []
            for k in range(KT):
                t = pt(f"xT{k}", [P, S], bf16)
                nc.sync.dma_start(out=t, in_=xT[k * P:(k + 1) * P, :])
                xTs.append(t)
            wqks = []
            for k in range(KT):
                t = pt(f"wqk{k}", [P, 2 * HB], bf16)
                nc.sync.dma_start(out=t, in_=wqkT[k * P:(k + 1) * P, :])
                wqks.append(t)
            wv1s = []
            for k in range(KT):
                t = pt(f"wv1{k}", [P, HPC * 65], bf16)
                nc.sync.dma_start(out=t, in_=wv1T[k * P:(k + 1) * P, :])
                wv1s.append(t)
            wos = []
            for k in range(KT):
                t = pt(f"wo{k}", [P, E], bf16)
                nc.sync.dma_start(out=t, in_=woT[k * P:(k + 1) * P, :])
                wos.append(t)
            bqks = []
            for m in range(4):
                t = pt(f"bqk{m}", [P, 1], f32)
                nc.sync.dma_start(out=t, in_=bqk[m * P:(m + 1) * P, :])
                bqks.append(t)
            bv1s = pt("bv1", [1, HPC * 65], bf16)
            nc.sync.dma_start(out=bv1s, in_=bv1[:, :])
            bos = pt("bo", [1, E], bf16)
            nc.sync.dma_start(out=bos, in_=bo[:, :])
            mks = pt("mask", [P, P], f32)
            nc.sync.dma_start(out=mks, in_=maskT[:, :])
            ids = pt("ident", [P, P], bf16)
            nc.sync.dma_start(out=ids, in_=ident[:, :])
            ones1 = pt("ones1", [1, P], bf16)
            nc.vector.memset(ones1, 1.0)

            # ---- phase 1a: q/k projection, transposed ----
            # qkT[m][feat 128, S]: m 0-1 = q feats (4 heads x 64), 2-3 = k
            qkT = [pt(f"qkT{m}", [P, S], bf16) for m in range(4)]
            for m in range(4):
                for n in range(S // 512):
                    p = mm.tile([P, 512], f32, tag="mm")
                    for k in range(KT):
                        nc.tensor.matmul(
                            p, wqks[k][:, m * P:(m + 1) * P],
                            xTs[k][:, n * 512:(n + 1) * 512],
                            start=(k == 0), stop=(k == KT - 1))
                    nc.vector.tensor_scalar_add(
                        qkT[m][:, n * 512:(n + 1) * 512], p, bqks[m])

            # ---- phase 1b: v projection, untransposed, with ones col ----
            v1 = [pt(f"v1_{sm}", [P, HPC * 65], bf16) for sm in range(QT)]
            for sm in range(QT):
                p = mm.tile([P, HPC * 65], f32, tag="mm")
                for k in range(KT):
                    nc.tensor.matmul(
                        p, xTs[k][:, sm * P:(sm + 1) * P], wv1s[k],
                        start=(k == 0), stop=False)
                nc.tensor.matmul(p, ones1, bv1s, start=False, stop=True)
                nc.vector.tensor_copy(v1[sm], p)

            # ---- phase 2: attention (4 local heads, causal) ----
            for qi in range(QT):
                ao = aop.tile([P, HB], bf16, tag="ao")
                for hh in range(HPC):
                    qrow = 64 * (hh % 2)
                    qtile = qkT[hh // 2]
                    ktile = qkT[2 + hh // 2]
                    pacc = po.tile([P, 65], f32, tag="po")
                    for kt in range(qi + 1):
                        sc = ps.tile([P, P], f32, tag="ps")
                        nc.tensor.matmul(
                            sc,
                            ktile[qrow:qrow + 64, kt * P:(kt + 1) * P],
                            qtile[qrow:qrow + 64, qi * P:(qi + 1) * P],
                            start=True, stop=True)
                        pb = probs.tile([P, P], bf16, tag="pb")
                        if kt == qi:
                            sm = sbs.tile([P, P], f32, tag="sbs")
                            nc.vector.tensor_add(sm, sc, mks)
                            nc.scalar.activation(pb, sm, AF.Exp, scale=0.125)
                        else:
                            nc.scalar.activation(pb, sc, AF.Exp, scale=0.125)
                        nc.tensor.matmul(
                            pacc, pb, v1[kt][:, 65 * hh:65 * hh + 65],
                            start=(kt == 0), stop=(kt == qi))
                    rc = rcp.tile([P, 1], f32, tag="rc")
                    nc.vector.reciprocal(rc, pacc[:, 64:65])
                    nc.vector.tensor_scalar_mul(
                        ao[:, 64 * hh:64 * (hh + 1)], pacc[:, 0:64], rc)
                # transpose [q, e] -> [e, q] and stage for the AllToAll
                for h2 in range(2):
                    pt_ = ps.tile([P, P], bf16, tag="ps")
                    nc.tensor.transpose(pt_, ao[:, h2 * P:(h2 + 1) * P], ids)
                    ot = otp.tile([P, P], bf16, tag="ot")
                    nc.scalar.copy(ot, pt_)
                    r0 = HB * (qi // 2) + P * h2
                    c0 = P * (qi % 2)
                    nc.sync.dma_start(
                        out=a2a_in[r0:r0 + P, c0:c0 + P], in_=ot)

            # ---- AllToAll: head-sharded -> row-sharded ----
            nc.gpsimd.collective_compute(
                "AllToAll", mybir.AluOpType.bypass,
                replica_groups=[list(range(NCORES))],
                ins=[a2a_in[:, :]], outs=[a2a_out[:, :]])

            # ---- phase 3: output projection on owned rows ----
            oc = []
            for t8 in range(2 * KT):   # 2 batches x 8 e-tiles
                t = pt(f"oc{t8}", [P, SB], bf16)
                nc.sync.dma_start(out=t, in_=a2a_out[t8 * P:(t8 + 1) * P, :])
                oc.append(t)
            for bb in range(2):
                for rt in range(SB // P):
                    for nf in range(E // 512):
                        p = mm.tile([P, 512], f32, tag="mm")
                        for e in range(KT):
                            nc.tensor.matmul(
                                p, oc[KT * bb + e][:, rt * P:(rt + 1) * P],
                                wos[e][:, nf * 512:(nf + 1) * 512],
                                start=(e == 0), stop=False)
                        nc.tensor.matmul(
                            p, ones1, bos[:, nf * 512:(nf + 1) * 512],
                            start=False, stop=True)
                        yt = yp.tile([P, 512], f32, tag="yt")
                        nc.vector.tensor_copy(yt, p)
                        r0 = SB * bb + P * rt
                        nc.sync.dma_start(
                            out=y[r0:r0 + P, nf * 512:(nf + 1) * 512], in_=yt)

    nc.compile()
    return nc


def _prep_inputs(x, W_in, b_in, W_out, b_out):
    """Host-side shard prep: returns in_maps list for cores 0..7."""
    x = np.asarray(x, np.float32)
    W_in = np.asarray(W_in, np.float32)
    b_in = np.asarray(b_in, np.float32)
    W_out = np.asarray(W_out, np.float32)
    b_out = np.asarray(b_out, np.float32)

    b_v = b_in[2 * E:3 * E]                       # [1024] in E order
    bo_eff = (b_out + W_out @ b_v).astype(np.float32)

    maskT = np.zeros((128, 128), np.float32)
    kk, qq = np.meshgrid(np.arange(128), np.arange(128), indexing="ij")
    maskT[kk > qq] = MASK_NEG
    ident = np.eye(128, dtype=BF16)
    bo_b = bo_eff[None, :].astype(BF16)

    in_maps = []
    for core in range(NCORES):
        b = core // 4
        g = core % 4
        h0 = 4 * g
        qrows = slice(64 * h0, 64 * h0 + HB)
        krows = slice(E + 64 * h0, E + 64 * h0 + HB)
        xTc = np.ascontiguousarray(x[b].T).astype(BF16)
        wqkT = np.ascontiguousarray(
            np.concatenate([W_in[qrows], W_in[krows]], axis=0).T).astype(BF16)
        bqk = np.concatenate(
            [b_in[qrows], b_in[krows]])[:, None].astype(np.float32)
        wv1T = np.zeros((E, HPC * 65), np.float32)
        bv1 = np.zeros((1, HPC * 65), np.float32)
        for hh in range(HPC):
            vrows = slice(2 * E + 64 * (h0 + hh), 2 * E + 64 * (h0 + hh) + 64)
            wv1T[:, 65 * hh:65 * hh + 64] = W_in[vrows].T
            bv1[0, 65 * hh + 64] = 1.0
        in_maps.append({
            "xT": xTc,
            "wqkT": wqkT,
            "wv1T": wv1T.astype(BF16),
            "bqk": bqk,
            "bv1": bv1.astype(BF16),
            "woT": np.ascontiguousarray(W_out.T).astype(BF16),
            "bo": bo_b,
            "maskT": maskT,
            "ident": ident,
        })
    return in_maps


def _run(in_maps, trace=False):
    from concourse.bass_utils import run_bass_kernel_spmd
    if "nc" not in _CACHE:
        _CACHE["nc"] = _build()
    return run_bass_kernel_spmd(
        _CACHE["nc"], in_maps, core_ids=list(range(NCORES)), trace=trace)


def kernel(x, W_in, b_in, W_out, b_out):
    in_maps = _prep_inputs(x, W_in, b_in, W_out, b_out)
    res = _run(in_maps)
    out = np.empty((B, S, E), np.float32)
    for core in range(NCORES):
        yc = np.asarray(res.results[core]["y"], np.float32)
        j = core
        out[0, SB * j:SB * (j + 1), :] = yc[:SB]
        out[1, SB * j:SB * (j + 1), :] = yc[SB:]
    return out
